# revision 1
# baseline (speedup 1.0000x reference)
"""Trainium2 Bass kernel for chemprop-style MPNN (nn_Cmpd_d_MPNN_3917010174549).

Strategy (8 NeuronCores, data-parallel over bonds with replicated message table):
  - The directed-bond message table ([n_bonds, 300], fp16) is replicated in each
    core's DRAM.  Each core computes the new messages for its 1/8 shard of
    bonds and an AllGather rebuilds the full table on every core after each
    message-passing step (3 AllGathers total).
  - Per message-passing step each bond needs 7 gathered rows
    (6 composed neighbour indices a2b[b2a[b]] plus the reverse bond b2revb[b]).
    All gather indices are static, so they are composed on the host and the
    gathers run as batched SWDGE indirect DMAs (one per 4-tile chunk).
  - The neighbour sum runs on the vector engine (fp16), the directed message
    update is fused into one PE matmul over the stacked contraction
    [f_bonds(147) ; u(300)] @ [W_i ; W_h] with fp32 PSUM accumulation
    (u = sum(neigh) - rev, and inp = f_bonds @ W_i is recomputed instead of
    stored).  Transposes of u run on the PE via identity matmuls.
  - Readout: atoms are sharded molecule-aligned.  Per atom tile: gather 6 rows,
    sum, fused matmul [f_atoms;1;a_message] @ [W_o_atoms; b_o; W_o_hidden],
    relu; the ragged per-molecule mean pooling is a PE matmul with host-built
    one-hot matrices accumulating 128 molecules per PSUM bank, scaled by
    host-computed reciprocal counts.

kernel(**inputs) takes the full unsharded inputs and returns [10000, 300] f32.
"""

import math
from dataclasses import dataclass, field

import numpy as np


# ----------------------------------------------------------------------------
# Configuration
# ----------------------------------------------------------------------------
@dataclass(frozen=True)
class Cfg:
    NC: int = 8            # cores
    NB: int = 300001       # bonds (incl. padding bond 0)
    NA: int = 150001       # atoms (incl. padding atom 0)
    NM: int = 10000        # molecules
    H: int = 300           # hidden
    FB: int = 147          # bond feature dim
    FA: int = 133          # atom feature dim
    MAXNB: int = 6
    DEPTH: int = 3
    CH: int = 8            # bond tiles per gather chunk
    CHR: int = 8           # readout tiles per gather chunk

    @property
    def Bs(self):          # bonds per core (multiple of 128)
        return ((self.NB + self.NC - 1) // self.NC + 127) // 128 * 128

    @property
    def NBP(self):
        return self.Bs * self.NC

    @property
    def NT_B(self):        # bond tiles per core
        return self.Bs // 128

    @property
    def Mc(self):          # molecules per core
        assert self.NM % self.NC == 0
        return self.NM // self.NC

    @property
    def G_n(self):         # 128-molecule groups per core
        return (self.Mc + 127) // 128


FULL = Cfg()

_BUILD_CACHE: dict = {}


# ----------------------------------------------------------------------------
# Host-side preprocessing
# ----------------------------------------------------------------------------
def prepare_host(inputs: dict, cfg: Cfg):
    """Build per-core input maps. Returns (in_maps, T_g)."""
    f_atoms = np.asarray(inputs["f_atoms"], np.float32)
    f_bonds = np.asarray(inputs["f_bonds"], np.float32)
    a2b = np.asarray(inputs["a2b"], np.int32)
    b2a = np.asarray(inputs["b2a"], np.int32)
    b2revb = np.asarray(inputs["b2revb"], np.int32)
    amol = np.asarray(inputs["atom_mol_id"], np.int32)
    W_i = np.asarray(inputs["W_i"], np.float32)
    W_h = np.asarray(inputs["W_h"], np.float32)
    W_o = np.asarray(inputs["W_o"], np.float32)
    b_o = np.asarray(inputs["b_o"], np.float32)

    NC, NB, NA, NM, H, FB, FA = (
        cfg.NC, cfg.NB, cfg.NA, cfg.NM, cfg.H, cfg.FB, cfg.FA)
    Bs, NBP, NT_B, Mc, G_n = cfg.Bs, cfg.NBP, cfg.NT_B, cfg.Mc, cfg.G_n

    # ---- bond side ----
    # composed gather indices: 7 table rows per bond (6 neighbours + reverse)
    nbr7 = np.zeros((NBP, 7), np.int32)
    nbr7[:NB, :6] = a2b[b2a]          # a2b[b2a[b], j]
    nbr7[:NB, 6] = b2revb

    # padded bond features, transposed per core
    fbT_full = np.zeros((FB, NBP), np.float16)
    fbT_full[:, :NB] = f_bonds.T.astype(np.float16)

    # ---- atom / molecule side ----
    # atoms (excluding padding atom 0) are sorted by molecule id
    mol_of_atom = amol
    # counts per molecule (excluding the dummy segment NM)
    counts = np.bincount(mol_of_atom, minlength=NM + 1)[:NM]
    # start offset of each molecule in the sorted atom order (atoms 1..NA-1)
    atom_ids = np.arange(NA, dtype=np.int64)
    valid = atom_ids != 0
    order = atom_ids[valid][np.argsort(mol_of_atom[valid], kind="stable")]
    mol_sorted = mol_of_atom[order]
    # group atoms: per core c, group g covers molecules [c*Mc + g*128, ...)
    starts = np.searchsorted(mol_sorted, np.arange(NM + 1))

    # tiles needed per (core, group)
    def group_atom_count(c, g):
        m0 = c * Mc + g * 128
        m1 = min(c * Mc + (g + 1) * 128, (c + 1) * Mc)
        return starts[m1] - starts[m0]

    T_g = 1
    for c in range(NC):
        for g in range(G_n):
            T_g = max(T_g, (group_atom_count(c, g) + 127) // 128)
    NRT = G_n * T_g                     # readout tiles per core

    inv_counts = np.zeros(NM, np.float32)
    nz = counts > 0
    inv_counts[nz] = 1.0 / counts[nz]

    faT_all = np.vstack([f_atoms.T.astype(np.float16),
                         np.ones((1, NA), np.float16)])  # [FA+1, NA]

    in_maps = []
    for c in range(NC):
        # bond inputs
        sl = slice(c * Bs, (c + 1) * Bs)
        idxb = (nbr7[sl]                       # [Bs, 7]
                .reshape(NT_B, 128, 7)
                .transpose(1, 0, 2)
                .reshape(128, NT_B * 7)
                .copy())
        fbT = np.ascontiguousarray(fbT_full[:, sl])

        # readout inputs
        slot_atom = np.zeros((NRT * 128,), np.int64)   # atom id per slot
        slot_valid = np.zeros((NRT * 128,), bool)
        oneh = np.zeros((NRT * 128, 128), np.float16)
        for g in range(G_n):
            m0 = c * Mc + g * 128
            m1 = min(c * Mc + (g + 1) * 128, (c + 1) * Mc)
            a0, a1 = starts[m0], starts[m1]
            n = a1 - a0
            base = g * T_g * 128
            slot_atom[base:base + n] = order[a0:a1]
            slot_valid[base:base + n] = True
            oneh[np.arange(base, base + n),
                 mol_sorted[a0:a1] - m0] = 1.0
        idxa = a2b[slot_atom, :6].astype(np.int32)     # [NRT*128, 6]
        idxa[~slot_valid] = 0
        idxa = (idxa.reshape(NRT, 128, 6)
                .transpose(1, 0, 2)
                .reshape(128, NRT * 6)
                .copy())
        faT = faT_all[:, slot_atom].copy()             # [FA+1, NRT*128]
        faT[:, ~slot_valid] = 0

        invc = np.zeros((128, G_n), np.float32)
        for g in range(G_n):
            m0 = c * Mc + g * 128
            m1 = min(c * Mc + (g + 1) * 128, (c + 1) * Mc)
            invc[: m1 - m0, g] = inv_counts[m0:m1]

        wstk = np.vstack([W_i, W_h]).astype(np.float16)            # [FB+H, H]
        wr = np.vstack([W_o[:FA], b_o[None, :], W_o[FA:]]).astype(np.float16)

        in_maps.append({
            "fbT": fbT, "idxb": idxb,
            "faT": faT, "idxa": idxa, "oneh": oneh,
            "wstk": wstk, "wr": wr, "invc": invc,
        })
    return in_maps, T_g


# ----------------------------------------------------------------------------
# Device program
# ----------------------------------------------------------------------------
def build_program(cfg: Cfg, T_g: int):
    import concourse.bass as bass
    import concourse.mybir as mybir
    import concourse.tile as tile
    from concourse import bacc
    from concourse.masks import make_identity

    dt = mybir.dt
    NC, H, FB, FA = cfg.NC, cfg.H, cfg.FB, cfg.FA
    Bs, NBP, NT_B, G_n = cfg.Bs, cfg.NBP, cfg.NT_B, cfg.G_n
    NRT = G_n * T_g
    CH, CHR = cfg.CH, cfg.CHR
    AluOp = mybir.AluOpType
    ActF = mybir.ActivationFunctionType
    RG = [list(range(NC))]

    # hidden-dim K chunks for the transposed operand (128/128/44 for H=300)
    h_chunks = []
    o = 0
    while o < H:
        h_chunks.append((o, min(o + 128, H)))
        o += 128
    n_hc = len(h_chunks)

    nc = bacc.Bacc(
        "TRN2", target_bir_lowering=False, debug=False,
        enable_asserts=False, num_devices=NC,
    )

    fbT = nc.dram_tensor("fbT", [FB, Bs], dt.float16, kind="ExternalInput").ap()
    idxb = nc.dram_tensor("idxb", [128, NT_B * 7], dt.int32,
                          kind="ExternalInput").ap()
    faT = nc.dram_tensor("faT", [FA + 1, NRT * 128], dt.float16,
                         kind="ExternalInput").ap()
    idxa = nc.dram_tensor("idxa", [128, NRT * 6], dt.int32,
                          kind="ExternalInput").ap()
    oneh = nc.dram_tensor("oneh", [NRT * 128, 128], dt.float16,
                          kind="ExternalInput").ap()
    wstk = nc.dram_tensor("wstk", [FB + H, H], dt.float16,
                          kind="ExternalInput").ap()
    wr = nc.dram_tensor("wr", [FA + 1 + H, H], dt.float16,
                        kind="ExternalInput").ap()
    invc = nc.dram_tensor("invc", [128, G_n], dt.float32,
                          kind="ExternalInput").ap()
    mol_out = nc.dram_tensor("mol_out", [G_n * 128, H], dt.float32,
                             kind="ExternalOutput").ap()

    with tile.TileContext(nc) as tc:
        with (
            tc.tile_pool(name="const", bufs=1) as cpool,
            tc.tile_pool(name="dram", bufs=1, space="DRAM") as dpool,
            tc.tile_pool(name="x", bufs=2) as xpool,
            tc.tile_pool(name="g", bufs=2) as gpool,
            tc.tile_pool(name="u", bufs=2) as upool,
            tc.tile_pool(name="ut", bufs=3) as utpool,
            tc.tile_pool(name="m", bufs=2) as mpool,
            tc.tile_pool(name="pt", bufs=2, space="PSUM") as ptpool,
            tc.tile_pool(name="po", bufs=2, space="PSUM") as popool,
            tc.tile_pool(name="pm", bufs=2, space="PSUM") as pmpool,
        ):
            # ---------------- DRAM internals ----------------
            msgs = [dpool.tile([NBP, H], dt.float16, addr_space="Shared",
                               name=f"msg{s}") for s in range(cfg.DEPTH)]
            shards = [dpool.tile([Bs, H], dt.float16, name=f"shard{s}")
                      for s in range(cfg.DEPTH)]

            # ---------------- constants ----------------
            ident = cpool.tile([128, 128], dt.float16, name="ident")
            make_identity(nc, ident[:, :])

            def load_chunks(src_ap, total_rows, name):
                tiles = []
                r = 0
                while r < total_rows:
                    rr = min(r + 128, total_rows)
                    t = cpool.tile([rr - r, H], dt.float16,
                                   name=f"{name}{len(tiles)}")
                    nc.sync.dma_start(out=t[:, :], in_=src_ap[r:rr, :])
                    tiles.append(t)
                    r = rr
                return tiles

            w_x = load_chunks(wstk, FB, "wx")           # W_i chunks
            w_h = [cpool.tile([b - a, H], dt.float16, name=f"wh{i}")
                   for i, (a, b) in enumerate(h_chunks)]
            for i, (a, b) in enumerate(h_chunks):
                nc.sync.dma_start(out=w_h[i][:, :], in_=wstk[FB + a:FB + b, :])
            wr_x = load_chunks(wr, FA + 1, "wrx")       # atom-side chunks
            wr_h = [cpool.tile([b - a, H], dt.float16, name=f"wrh{i}")
                    for i, (a, b) in enumerate(h_chunks)]
            for i, (a, b) in enumerate(h_chunks):
                nc.sync.dma_start(out=wr_h[i][:, :],
                                  in_=wr[FA + 1 + a:FA + 1 + b, :])

            idxb_sb = cpool.tile([128, NT_B * 7], dt.int32, name="idxb_sb")
            nc.sync.dma_start(out=idxb_sb[:, :], in_=idxb[:, :])
            idxa_sb = cpool.tile([128, NRT * 6], dt.int32, name="idxa_sb")
            nc.sync.dma_start(out=idxa_sb[:, :], in_=idxa[:, :])
            invc_sb = cpool.tile([128, G_n], dt.float32, name="invc_sb")
            nc.sync.dma_start(out=invc_sb[:, :], in_=invc[:, :])

            # ---------------- helpers ----------------
            def load_x(src, width, t0, ch_t, tagbase):
                """Load [width, ch_t*128] K-major feature slab as <=128-row tiles."""
                tiles = []
                r = 0
                i = 0
                while r < width:
                    rr = min(r + 128, width)
                    xt = xpool.tile([rr - r, ch_t * 128], dt.float16,
                                    tag=f"{tagbase}{i}", name=f"{tagbase}{i}")
                    nc.sync.dma_start(
                        out=xt[:, :],
                        in_=src[r:rr, t0 * 128:(t0 + ch_t) * 128])
                    tiles.append(xt)
                    r = rr
                    i += 1
                return tiles

            def transpose_u(u_slice_fn):
                """PE-transpose H columns of a [128, H] fp16 operand into a
                [128, n_hc*128] fp16 SBUF tile (chunk i at cols i*128)."""
                pt = ptpool.tile([128, n_hc * 128], dt.float16, tag="pt",
                                 name="pt")
                uT = utpool.tile([128, n_hc * 128], dt.float16, tag="uT",
                                 name="uT")
                for i, (a, b) in enumerate(h_chunks):
                    w = b - a
                    nc.tensor.transpose(
                        out=pt[0:w, i * 128:i * 128 + 128],
                        in_=u_slice_fn(a, b),
                        identity=ident[:, :])
                    nc.vector.tensor_copy(
                        out=uT[0:w, i * 128:i * 128 + 128],
                        in_=pt[0:w, i * 128:i * 128 + 128])
                return uT

            def shard_view(shard, t0, ch_t):
                return shard[t0 * 128:(t0 + ch_t) * 128, :].rearrange(
                    "(c p) h -> p c h", p=128)

            # ---------------- message passing steps ----------------
            for step in range(cfg.DEPTH):
                src = None if step == 0 else msgs[step - 1]
                dst_shard = shards[step]
                for t0 in range(0, NT_B, CH):
                    ch_t = min(CH, NT_B - t0)
                    x_tiles = load_x(fbT, FB, t0, ch_t, "x")
                    if step > 0:
                        G3 = gpool.tile([128, ch_t * 7, H], dt.float16,
                                        tag="g", name="G3")
                        # HW only supports one offset per partition per
                        # indirect DMA -> one instruction per (tile, slot)
                        for t in range(ch_t):
                            for j in range(7):
                                nc.gpsimd.indirect_dma_start(
                                    out=G3[:, t * 7 + j, :],
                                    out_offset=None,
                                    in_=src[:, :],
                                    in_offset=bass.IndirectOffsetOnAxis(
                                        ap=idxb_sb[:, (t0 + t) * 7 + j:
                                                   (t0 + t) * 7 + j + 1],
                                        axis=0),
                                )
                        G = G3[:, :, :].rearrange("p (c s) h -> p c s h", s=7)
                        U = upool.tile([128, ch_t, H], dt.float16, tag="u",
                                       name="U")
                        T2 = upool.tile([128, ch_t, H], dt.float16, tag="t2",
                                        name="T2")
                        tt = nc.vector.tensor_tensor
                        tt(out=U[:, :, :], in0=G[:, :, 0, :],
                           in1=G[:, :, 1, :], op=AluOp.add)
                        tt(out=T2[:, :, :], in0=G[:, :, 2, :],
                           in1=G[:, :, 3, :], op=AluOp.add)
                        tt(out=U[:, :, :], in0=U[:, :, :],
                           in1=T2[:, :, :], op=AluOp.add)
                        tt(out=T2[:, :, :], in0=G[:, :, 4, :],
                           in1=G[:, :, 5, :], op=AluOp.add)
                        tt(out=U[:, :, :], in0=U[:, :, :],
                           in1=T2[:, :, :], op=AluOp.add)
                        tt(out=U[:, :, :], in0=U[:, :, :],
                           in1=G[:, :, 6, :], op=AluOp.subtract)

                    msg_sb = mpool.tile([128, ch_t, H], dt.float16, tag="msg",
                                        name="msg_sb")
                    for t in range(ch_t):
                        po = popool.tile([128, H], dt.float32, tag="po",
                                         name="po")
                        n_mm = len(w_x) + (n_hc if step > 0 else 0)
                        k = 0
                        r = 0
                        for xt, wt in zip(x_tiles, w_x):
                            nc.tensor.matmul(
                                out=po[:, :], lhsT=xt[:, t * 128:(t + 1) * 128],
                                rhs=wt[:, :],
                                start=(k == 0), stop=(k == n_mm - 1))
                            k += 1
                        if step > 0:
                            uT = transpose_u(
                                lambda a, b: U[:, t, a:b])
                            for i, (a, b) in enumerate(h_chunks):
                                w = b - a
                                nc.tensor.matmul(
                                    out=po[:, :],
                                    lhsT=uT[0:w, i * 128:i * 128 + 128],
                                    rhs=w_h[i][:, :],
                                    start=(k == 0), stop=(k == n_mm - 1))
                                k += 1
                        nc.scalar.activation(
                            out=msg_sb[:, t, :], in_=po[:, :], func=ActF.Relu)
                    nc.sync.dma_start(out=shard_view(dst_shard, t0, ch_t),
                                      in_=msg_sb[:, :, :])
                # AllGather the new message shard into the full table
                nc.gpsimd.collective_compute(
                    "AllGather", AluOp.bypass, replica_groups=RG,
                    ins=[dst_shard[:, :]], outs=[msgs[step][:, :]],
                )

            # ---------------- readout ----------------
            src = msgs[cfg.DEPTH - 1]
            for g in range(G_n):
                pm = pmpool.tile([128, H], dt.float32, tag="pm", name="pm")
                for t0 in range(0, T_g, CHR):
                    ch_t = min(CHR, T_g - t0)
                    rt0 = g * T_g + t0
                    Ga3 = gpool.tile([128, ch_t * 6, H], dt.float16, tag="g",
                                     name="Ga3")
                    for t in range(ch_t):
                        for j in range(6):
                            nc.gpsimd.indirect_dma_start(
                                out=Ga3[:, t * 6 + j, :],
                                out_offset=None,
                                in_=src[:, :],
                                in_offset=bass.IndirectOffsetOnAxis(
                                    ap=idxa_sb[:, (rt0 + t) * 6 + j:
                                               (rt0 + t) * 6 + j + 1],
                                    axis=0),
                            )
                    Ga = Ga3[:, :, :].rearrange("p (c s) h -> p c s h", s=6)
                    U = upool.tile([128, ch_t, H], dt.float16, tag="u",
                                   name="Ua")
                    T2 = upool.tile([128, ch_t, H], dt.float16, tag="t2",
                                    name="T2a")
                    tt = nc.vector.tensor_tensor
                    tt(out=U[:, :, :], in0=Ga[:, :, 0, :], in1=Ga[:, :, 1, :],
                       op=AluOp.add)
                    tt(out=T2[:, :, :], in0=Ga[:, :, 2, :], in1=Ga[:, :, 3, :],
                       op=AluOp.add)
                    tt(out=U[:, :, :], in0=U[:, :, :], in1=T2[:, :, :],
                       op=AluOp.add)
                    tt(out=T2[:, :, :], in0=Ga[:, :, 4, :], in1=Ga[:, :, 5, :],
                       op=AluOp.add)
                    tt(out=U[:, :, :], in0=U[:, :, :], in1=T2[:, :, :],
                       op=AluOp.add)

                    xa_tiles = load_x(faT, FA + 1, rt0, ch_t, "xa")
                    for t in range(ch_t):
                        tg = t0 + t          # tile index within group
                        ph = popool.tile([128, H], dt.float32, tag="po",
                                         name="ph")
                        n_mm = len(wr_x) + n_hc
                        k = 0
                        for xt, wt in zip(xa_tiles, wr_x):
                            nc.tensor.matmul(
                                out=ph[:, :], lhsT=xt[:, t * 128:(t + 1) * 128],
                                rhs=wt[:, :],
                                start=(k == 0), stop=(k == n_mm - 1))
                            k += 1
                        uT = transpose_u(lambda a, b: U[:, t, a:b])
                        for i, (a, b) in enumerate(h_chunks):
                            w = b - a
                            nc.tensor.matmul(
                                out=ph[:, :],
                                lhsT=uT[0:w, i * 128:i * 128 + 128],
                                rhs=wr_h[i][:, :],
                                start=(k == 0), stop=(k == n_mm - 1))
                            k += 1
                        ah = mpool.tile([128, H], dt.float16, tag="ah",
                                        name="ah")
                        nc.scalar.activation(out=ah[:, :], in_=ph[:, :],
                                             func=ActF.Relu)
                        oh = mpool.tile([128, 128], dt.float16, tag="oh",
                                        name="oh")
                        rt = g * T_g + tg
                        nc.sync.dma_start(
                            out=oh[:, :],
                            in_=oneh[rt * 128:(rt + 1) * 128, :])
                        nc.tensor.matmul(
                            out=pm[:, :], lhsT=oh[:, :], rhs=ah[:, :],
                            start=(tg == 0), stop=(tg == T_g - 1))
                out_sb = mpool.tile([128, H], dt.float32, tag="osb",
                                    name="out_sb")
                nc.vector.tensor_scalar_mul(
                    out=out_sb[:, :], in0=pm[:, :],
                    scalar1=invc_sb[:, g:g + 1])
                nc.sync.dma_start(out=mol_out[g * 128:(g + 1) * 128, :],
                                  in_=out_sb[:, :])

    nc.compile()
    return nc


# ----------------------------------------------------------------------------
# Entry point
# ----------------------------------------------------------------------------
def _get_program(cfg: Cfg, T_g: int):
    key = (cfg, T_g)
    if key not in _BUILD_CACHE:
        _BUILD_CACHE[key] = build_program(cfg, T_g)
    return _BUILD_CACHE[key]


def run_on_hw(nc, in_maps, cfg, trace=False):
    from concourse.bass_utils import run_bass_kernel_spmd
    res = run_bass_kernel_spmd(nc, in_maps, list(range(cfg.NC)), trace=trace)
    return res


def assemble_output(results, cfg: Cfg):
    out = np.zeros((cfg.NM, cfg.H), np.float32)
    for c in range(cfg.NC):
        out[c * cfg.Mc:(c + 1) * cfg.Mc] = results[c]["mol_out"][:cfg.Mc]
    return out


def kernel(**inputs) -> np.ndarray:
    cfg = FULL
    in_maps, T_g = prepare_host(inputs, cfg)
    nc = _get_program(cfg, T_g)
    res = run_on_hw(nc, in_maps, cfg)
    return assemble_output(res.results, cfg)



# revision 5
# speedup vs baseline: 1.0281x; 1.0281x over previous
"""Trainium2 Bass kernel for chemprop-style MPNN (nn_Cmpd_d_MPNN_3917010174549).

Strategy (8 NeuronCores, data-parallel with replicated tables):
  - Directed-bond message table msg [NBP, 300] fp16 replicated per core via
    chunked AllGathers (overlapped with compute).
  - Atom-sum (S) formulation: per step, S[a] = sum_j msg[a2b[a,j]] is computed
    once per atom (sharded molecule-aligned, AllGathered), then the bond
    update needs only 2 gathers per bond: u = S[b2a[b]] - msg[b2revb[b]].
    This cuts SWDGE indirect-DMA instructions (the bottleneck: ~1us fixed
    cost each) from 7 to ~5.3 per bond-tile equivalent.
  - inp = f_bonds @ W_i is computed once in step 0 and stored (fp16) in DRAM;
    steps 1-2 stream it instead of recomputing the W_i matmul.
  - Readout reuses the atom-phase: S2 chunks are consumed directly from SBUF
    into the fused [f_atoms;1;S2] @ [W_o;b_o] matmul and one-hot mean pooling.

kernel(**inputs) takes the full unsharded inputs and returns [10000, 300] f32.
"""

from dataclasses import dataclass

import numpy as np


# ----------------------------------------------------------------------------
# Configuration
# ----------------------------------------------------------------------------
@dataclass(frozen=True)
class Cfg:
    NC: int = 8            # cores
    NB: int = 300001       # bonds (incl. padding bond 0)
    NA: int = 150001       # atoms (incl. padding atom 0)
    NM: int = 10000        # molecules
    H: int = 300           # hidden
    FB: int = 147          # bond feature dim
    FA: int = 133          # atom feature dim
    MAXNB: int = 6
    DEPTH: int = 3
    CH: int = 8            # bond tiles per chunk
    CHA: int = 8           # atom-slot tiles per chunk
    N_AGM: int = 1         # AllGather chunks for the message table
    N_AGS: int = 1         # AllGather chunks for the S table

    @property
    def Bs(self):          # bonds per core (multiple of 128)
        return ((self.NB + self.NC - 1) // self.NC + 127) // 128 * 128

    @property
    def NBP(self):
        return self.Bs * self.NC

    @property
    def NT_B(self):        # bond tiles per core
        return self.Bs // 128

    @property
    def Mc(self):          # molecules per core
        assert self.NM % self.NC == 0
        return self.NM // self.NC

    @property
    def G_n(self):         # 128-molecule groups per core
        return (self.Mc + 127) // 128


FULL = Cfg()

_BUILD_CACHE: dict = {}


def _chunk_tiles(n_tiles, n_chunks):
    """Split n_tiles into n_chunks tile-counts (near equal)."""
    base = n_tiles // n_chunks
    rem = n_tiles % n_chunks
    return [base + (1 if i < rem else 0) for i in range(n_chunks)]


# ----------------------------------------------------------------------------
# Host-side preprocessing
# ----------------------------------------------------------------------------
def prepare_host(inputs: dict, cfg: Cfg):
    """Build per-core input maps. Returns (in_maps, T_g)."""
    f_atoms = np.asarray(inputs["f_atoms"], np.float32)
    f_bonds = np.asarray(inputs["f_bonds"], np.float32)
    a2b = np.asarray(inputs["a2b"], np.int32)
    b2a = np.asarray(inputs["b2a"], np.int32)
    b2revb = np.asarray(inputs["b2revb"], np.int32)
    amol = np.asarray(inputs["atom_mol_id"], np.int32)
    W_i = np.asarray(inputs["W_i"], np.float32)
    W_h = np.asarray(inputs["W_h"], np.float32)
    W_o = np.asarray(inputs["W_o"], np.float32)
    b_o = np.asarray(inputs["b_o"], np.float32)

    NC, NB, NA, NM, H, FB, FA = (
        cfg.NC, cfg.NB, cfg.NA, cfg.NM, cfg.H, cfg.FB, cfg.FA)
    Bs, NT_B, Mc, G_n = cfg.Bs, cfg.NT_B, cfg.Mc, cfg.G_n

    # ---- molecule-aligned atom slots (shared by steps' S phase + readout) ----
    mol_of_atom = amol
    counts = np.bincount(mol_of_atom, minlength=NM + 1)[:NM]
    atom_ids = np.arange(NA, dtype=np.int64)
    valid = atom_ids != 0
    order = atom_ids[valid][np.argsort(mol_of_atom[valid], kind="stable")]
    mol_sorted = mol_of_atom[order]
    starts = np.searchsorted(mol_sorted, np.arange(NM + 1))

    def group_atom_count(c, g):
        m0 = c * Mc + g * 128
        m1 = min(c * Mc + (g + 1) * 128, (c + 1) * Mc)
        return starts[m1] - starts[m0]

    T_g = 1
    for c in range(NC):
        for g in range(G_n):
            T_g = max(T_g, (group_atom_count(c, g) + 127) // 128)
    NRT = G_n * T_g                     # slot tiles per core
    SLOTS = NRT * 128                   # atom slots per core

    inv_counts = np.zeros(NM, np.float32)
    nz = counts > 0
    inv_counts[nz] = 1.0 / counts[nz]

    # slot assignment per core
    slot_atom = np.zeros((NC, SLOTS), np.int64)
    slot_valid = np.zeros((NC, SLOTS), bool)
    for c in range(NC):
        for g in range(G_n):
            m0 = c * Mc + g * 128
            m1 = min(c * Mc + (g + 1) * 128, (c + 1) * Mc)
            a0, a1 = starts[m0], starts[m1]
            n = a1 - a0
            base = g * T_g * 128
            slot_atom[c, base:base + n] = order[a0:a1]
            slot_valid[c, base:base + n] = True
    # atom 0 (padding atom, but queried via b2a) -> first free slot on core 0
    free0 = np.where(~slot_valid[0])[0]
    assert len(free0) > 0, "no free slot for atom 0"
    a0slot = free0[0]
    slot_atom[0, a0slot] = 0
    slot_valid[0, a0slot] = True

    # global S-table row per atom (S table layout: [chunk][core][rows])
    s_chunks = _chunk_tiles(NRT, cfg.N_AGS)
    s_off = np.cumsum([0] + s_chunks[:-1]) * 128      # local row offsets
    s_csz = np.array(s_chunks) * 128
    s_gbase = np.cumsum([0] + list(s_csz[:-1] * NC))  # global chunk bases

    def srow_of(c, s):
        """Global S-table row for (core c, slot s). Vectorized over s."""
        j = np.searchsorted(s_off, s, side="right") - 1
        return s_gbase[j] + c * s_csz[j] + (s - s_off[j])

    srow_atom = np.zeros(NA, np.int64)
    for c in range(NC):
        sl = np.where(slot_valid[c])[0]
        srow_atom[slot_atom[c, sl]] = srow_of(c, sl)

    # ---- bond shard + new global message-row numbering ----
    # core c owns original bonds [c*Bs, (c+1)*Bs) (with padding at the end);
    # message table layout: [chunk][core][local rows]
    m_chunks = _chunk_tiles(NT_B, cfg.N_AGM)
    m_off = np.cumsum([0] + m_chunks[:-1]) * 128
    m_csz = np.array(m_chunks) * 128
    m_gbase = np.cumsum([0] + list(m_csz[:-1] * NC))

    local_r = np.arange(Bs, dtype=np.int64)
    jj = np.searchsorted(m_off, local_r, side="right") - 1
    newrow_local = m_gbase[jj] + (local_r - m_off[jj])  # + c*m_csz[j]
    newrow = np.zeros(cfg.NBP, np.int64)
    for c in range(NC):
        o = np.minimum((c + 1) * Bs, cfg.NBP)
        n = o - c * Bs
        newrow[c * Bs:o] = newrow_local[:n] + (m_csz[jj[:n]] * c)
    # original bond id -> new global msg row
    msgrow_of_bond = newrow[:NB]

    # ---- per-core index arrays ----
    # A-phase queries: for slot tile t, neighbor j: msg row of a2b[atom,j]
    # B-phase: srow(b2a[b]) and msgrow(b2revb[b]) per bond
    wstk_h = W_h.astype(np.float16)                    # [H, H]
    wi = W_i.astype(np.float16)                        # [FB, H]
    wr = np.vstack([W_o[:FA], b_o[None, :], W_o[FA:]]).astype(np.float16)

    faT_all = np.vstack([f_atoms.T.astype(np.float16),
                         np.ones((1, NA), np.float16)])  # [FA+1, NA]

    in_maps = []
    for c in range(NC):
        sl = slice(c * Bs, min((c + 1) * Bs, cfg.NBP))
        n_real = sl.stop - sl.start

        # bond features, transposed, padded
        fbT = np.zeros((FB, Bs), np.float16)
        fbT[:, :min(n_real, NB - c * Bs) if c * Bs < NB else 0] = (
            f_bonds[c * Bs:min((c + 1) * Bs, NB)].T.astype(np.float16))

        # A-phase idx: [128, NRT*6]
        sa = slot_atom[c]                               # [SLOTS]
        qa = msgrow_of_bond[
            np.where(slot_valid[c][:, None], a2b[sa, :6], 0)]  # [SLOTS, 6]
        idxa = (qa.reshape(NRT, 128, 6)
                .transpose(1, 0, 2)
                .reshape(128, NRT * 6)
                .astype(np.int32).copy())

        # B-phase idx: [128, NT_B] each
        b_ids = np.arange(c * Bs, c * Bs + Bs, dtype=np.int64)
        b_ids = np.minimum(b_ids, NB - 1)               # pad bonds -> bond NB-1
        qs = srow_atom[b2a[b_ids]]                      # [Bs]
        qr = msgrow_of_bond[b2revb[b_ids]]              # [Bs]
        idxb_s = (qs.reshape(NT_B, 128).T.astype(np.int32).copy())
        idxb_r = (qr.reshape(NT_B, 128).T.astype(np.int32).copy())

        # readout aux
        oneh = np.zeros((NRT * 128, 128), np.float16)
        for g in range(G_n):
            m0 = c * Mc + g * 128
            m1 = min(c * Mc + (g + 1) * 128, (c + 1) * Mc)
            a0, a1 = starts[m0], starts[m1]
            n = a1 - a0
            base = g * T_g * 128
            oneh[np.arange(base, base + n),
                 mol_sorted[a0:a1] - m0] = 1.0
        faT = faT_all[:, slot_atom[c]].copy()           # [FA+1, SLOTS]
        fa_invalid = ~slot_valid[c]
        faT[:, fa_invalid] = 0
        if c == 0:
            faT[:, a0slot] = 0                          # atom 0: no readout

        invc = np.zeros((128, G_n), np.float32)
        for g in range(G_n):
            m0 = c * Mc + g * 128
            m1 = min(c * Mc + (g + 1) * 128, (c + 1) * Mc)
            invc[: m1 - m0, g] = inv_counts[m0:m1]

        in_maps.append({
            "fbT": fbT,
            "idxa": idxa, "idxb_s": idxb_s, "idxb_r": idxb_r,
            "faT": faT, "oneh": oneh, "invc": invc,
            "wi": wi, "wh": wstk_h, "wr": wr,
        })
    return in_maps, T_g


# ----------------------------------------------------------------------------
# Device program
# ----------------------------------------------------------------------------
def build_program(cfg: Cfg, T_g: int):
    import concourse.bass as bass
    import concourse.mybir as mybir
    import concourse.tile as tile
    from concourse import bacc
    from concourse.masks import make_identity

    dt = mybir.dt
    NC, H, FB, FA = cfg.NC, cfg.H, cfg.FB, cfg.FA
    Bs, NBP, NT_B = cfg.Bs, cfg.NBP, cfg.NT_B
    G_n = cfg.G_n
    NRT = G_n * T_g
    SLOTS = NRT * 128
    CH, CHA = cfg.CH, cfg.CHA
    AluOp = mybir.AluOpType
    ActF = mybir.ActivationFunctionType
    RG = [list(range(NC))]

    m_chunks = _chunk_tiles(NT_B, cfg.N_AGM)     # msg AG chunks (tiles)
    s_chunks = _chunk_tiles(NRT, cfg.N_AGS)      # S AG chunks (tiles)

    h_chunks = []
    o = 0
    while o < H:
        h_chunks.append((o, min(o + 128, H)))
        o += 128
    n_hc = len(h_chunks)

    fb_chunks = []
    o = 0
    while o < FB:
        fb_chunks.append((o, min(o + 128, FB)))
        o += 128

    fa_chunks = []
    o = 0
    while o < FA + 1:
        fa_chunks.append((o, min(o + 128, FA + 1)))
        o += 128

    nc = bacc.Bacc(
        "TRN2", target_bir_lowering=False, debug=False,
        enable_asserts=False, num_devices=NC,
    )

    fbT = nc.dram_tensor("fbT", [FB, Bs], dt.float16, kind="ExternalInput").ap()
    idxa = nc.dram_tensor("idxa", [128, NRT * 6], dt.int32,
                          kind="ExternalInput").ap()
    idxb_s = nc.dram_tensor("idxb_s", [128, NT_B], dt.int32,
                            kind="ExternalInput").ap()
    idxb_r = nc.dram_tensor("idxb_r", [128, NT_B], dt.int32,
                            kind="ExternalInput").ap()
    faT = nc.dram_tensor("faT", [FA + 1, SLOTS], dt.float16,
                         kind="ExternalInput").ap()
    oneh = nc.dram_tensor("oneh", [SLOTS, 128], dt.float16,
                          kind="ExternalInput").ap()
    invc = nc.dram_tensor("invc", [128, G_n], dt.float32,
                          kind="ExternalInput").ap()
    wi_d = nc.dram_tensor("wi", [FB, H], dt.float16, kind="ExternalInput").ap()
    wh_d = nc.dram_tensor("wh", [H, H], dt.float16, kind="ExternalInput").ap()
    wr_d = nc.dram_tensor("wr", [FA + 1 + H, H], dt.float16,
                          kind="ExternalInput").ap()
    mol_out = nc.dram_tensor("mol_out", [G_n * 128, H], dt.float32,
                             kind="ExternalOutput").ap()

    with tile.TileContext(nc) as tc:
        with (
            tc.tile_pool(name="const", bufs=1) as cpool,
            tc.tile_pool(name="dram", bufs=1, space="DRAM") as dpool,
            tc.tile_pool(name="x", bufs=2) as xpool,
            tc.tile_pool(name="g", bufs=2) as gpool,
            tc.tile_pool(name="u", bufs=2) as upool,
            tc.tile_pool(name="ut", bufs=3) as utpool,
            tc.tile_pool(name="m", bufs=2) as mpool,
            tc.tile_pool(name="i", bufs=2) as ipool,
            tc.tile_pool(name="pt", bufs=2, space="PSUM") as ptpool,
            tc.tile_pool(name="po", bufs=2, space="PSUM") as popool,
            tc.tile_pool(name="pm", bufs=2, space="PSUM") as pmpool,
        ):
            # ---------------- DRAM internals ----------------
            msgs = [dpool.tile([NBP, H], dt.float16, addr_space="Shared",
                               name=f"msg{s}") for s in range(cfg.DEPTH)]
            mshard = [dpool.tile([Bs, H], dt.float16, name=f"mshard{s}")
                      for s in range(cfg.DEPTH)]
            stab = [dpool.tile([SLOTS * NC, H], dt.float16,
                               addr_space="Shared", name=f"stab{s}")
                    for s in range(2)]
            sshard = [dpool.tile([SLOTS, H], dt.float16, name=f"sshard{s}")
                      for s in range(2)]
            urev_d = [dpool.tile([Bs, H], dt.float16, name=f"urev{s}")
                      for s in range(2)]
            inp_d = dpool.tile([Bs, H], dt.float16, name="inp_d")

            # ---------------- constants ----------------
            ident = cpool.tile([128, 128], dt.float16, name="ident")
            make_identity(nc, ident[:, :])

            w_i = []
            for i, (a, b) in enumerate(fb_chunks):
                t = cpool.tile([b - a, H], dt.float16, name=f"wi{i}")
                nc.sync.dma_start(out=t[:, :], in_=wi_d[a:b, :])
                w_i.append(t)
            w_h = []
            for i, (a, b) in enumerate(h_chunks):
                t = cpool.tile([b - a, H], dt.float16, name=f"wh{i}")
                nc.sync.dma_start(out=t[:, :], in_=wh_d[a:b, :])
                w_h.append(t)
            wr_x = []
            for i, (a, b) in enumerate(fa_chunks):
                t = cpool.tile([b - a, H], dt.float16, name=f"wrx{i}")
                nc.sync.dma_start(out=t[:, :], in_=wr_d[a:b, :])
                wr_x.append(t)
            wr_h = []
            for i, (a, b) in enumerate(h_chunks):
                t = cpool.tile([b - a, H], dt.float16, name=f"wrh{i}")
                nc.sync.dma_start(out=t[:, :], in_=wr_d[FA + 1 + a:FA + 1 + b, :])
                wr_h.append(t)

            idxa_sb = cpool.tile([128, NRT * 6], dt.int32, name="idxa_sb")
            nc.sync.dma_start(out=idxa_sb[:, :], in_=idxa[:, :])
            idxbs_sb = cpool.tile([128, NT_B], dt.int32, name="idxbs_sb")
            nc.sync.dma_start(out=idxbs_sb[:, :], in_=idxb_s[:, :])
            idxbr_sb = cpool.tile([128, NT_B], dt.int32, name="idxbr_sb")
            nc.sync.dma_start(out=idxbr_sb[:, :], in_=idxb_r[:, :])
            invc_sb = cpool.tile([128, G_n], dt.float32, name="invc_sb")
            nc.sync.dma_start(out=invc_sb[:, :], in_=invc[:, :])

            # ---------------- helpers ----------------
            def transpose_u(u_slice_fn):
                pt = ptpool.tile([128, n_hc * 128], dt.float16, tag="pt",
                                 name="pt")
                uT = utpool.tile([128, n_hc * 128], dt.float16, tag="uT",
                                 name="uT")
                for i, (a, b) in enumerate(h_chunks):
                    w = b - a
                    nc.tensor.transpose(
                        out=pt[0:w, i * 128:i * 128 + 128],
                        in_=u_slice_fn(a, b),
                        identity=ident[:, :])
                    nc.vector.tensor_copy(
                        out=uT[0:w, i * 128:i * 128 + 128],
                        in_=pt[0:w, i * 128:i * 128 + 128])
                return uT

            def shard_rows(shard, t0, ch_t):
                return shard[t0 * 128:(t0 + ch_t) * 128, :].rearrange(
                    "(c p) h -> p c h", p=128)

            def ag_chunks_of(chunks):
                """[(tile0, ntiles, local_row0, glob_row0), ...]"""
                out = []
                t0 = 0
                g0 = 0
                for ct in chunks:
                    out.append((t0, ct, t0 * 128, g0))
                    g0 += ct * 128 * NC
                    t0 += ct
                return out

            m_ag = ag_chunks_of(m_chunks)
            s_ag = ag_chunks_of(s_chunks)

            def ag_msg(step, agj):
                t0, ct, lr0, gr0 = m_ag[agj]
                nc.gpsimd.collective_compute(
                    "AllGather", AluOp.bypass, replica_groups=RG,
                    ins=[mshard[step][lr0:lr0 + ct * 128, :]],
                    outs=[msgs[step][gr0:gr0 + ct * 128 * NC, :]],
                )

            def ag_s(k, agj):
                t0, ct, lr0, gr0 = s_ag[agj]
                nc.gpsimd.collective_compute(
                    "AllGather", AluOp.bypass, replica_groups=RG,
                    ins=[sshard[k][lr0:lr0 + ct * 128, :]],
                    outs=[stab[k][gr0:gr0 + ct * 128 * NC, :]],
                )

            # ---------------- step 0: inp + msg0 ----------------
            for t0, ct_tiles, lr0, gr0 in m_ag:
                for c0 in range(t0, t0 + ct_tiles, CH):
                    ch_t = min(CH, t0 + ct_tiles - c0)
                    x_tiles = []
                    for i, (a, b) in enumerate(fb_chunks):
                        xt = xpool.tile([b - a, CH * 128], dt.float16,
                                        tag=f"x{i}", name=f"x{i}")
                        nc.sync.dma_start(
                            out=xt[:, :ch_t * 128],
                            in_=fbT[a:b, c0 * 128:(c0 + ch_t) * 128])
                        x_tiles.append(xt)
                    inp_sb = ipool.tile([128, CH, H], dt.float16, tag="inp0",
                                        name="inp_sb")
                    msg_sb = mpool.tile([128, CH, H], dt.float16, tag="msg",
                                        name="msg_sb")
                    for t in range(ch_t):
                        po = popool.tile([128, H], dt.float32, tag="po",
                                         name="po")
                        for k, (xt, wt) in enumerate(zip(x_tiles, w_i)):
                            nc.tensor.matmul(
                                out=po[:, :],
                                lhsT=xt[:, t * 128:(t + 1) * 128],
                                rhs=wt[:, :],
                                start=(k == 0), stop=(k == len(w_i) - 1))
                        nc.scalar.activation(
                            out=inp_sb[:, t, :], in_=po[:, :], func=ActF.Copy)
                        nc.scalar.activation(
                            out=msg_sb[:, t, :], in_=po[:, :], func=ActF.Relu)
                    nc.sync.dma_start(
                        out=shard_rows(inp_d, c0, ch_t),
                        in_=inp_sb[:, :ch_t, :])
                    nc.sync.dma_start(
                        out=shard_rows(mshard[0], c0, ch_t),
                        in_=msg_sb[:, :ch_t, :])
                ag_msg(0, m_ag.index((t0, ct_tiles, lr0, gr0)))

            # ---------------- steps 1..DEPTH-1 ----------------
            for step in range(1, cfg.DEPTH):
                k = step - 1          # S table index
                src = msgs[step - 1]
                # ---- A phase: S[slot] = sum_j msg[a2b] ----
                for t0, ct_tiles, lr0, gr0 in s_ag:
                    for c0 in range(t0, t0 + ct_tiles, CHA):
                        ch_t = min(CHA, t0 + ct_tiles - c0)
                        G3 = gpool.tile([128, CHA * 6, H], dt.float16,
                                        tag="g", name="G3")
                        for t in range(ch_t):
                            for j in range(6):
                                nc.gpsimd.indirect_dma_start(
                                    out=G3[:, t * 6 + j, :],
                                    out_offset=None,
                                    in_=src[:, :],
                                    in_offset=bass.IndirectOffsetOnAxis(
                                        ap=idxa_sb[:, (c0 + t) * 6 + j:
                                                   (c0 + t) * 6 + j + 1],
                                        axis=0),
                                )
                        Gv = G3[:, :ch_t * 6, :].rearrange(
                            "p (c s) h -> p c s h", s=6)
                        U = upool.tile([128, CHA, H], dt.float16, tag="u",
                                       name="U")
                        T2 = upool.tile([128, CHA, H], dt.float16, tag="t2",
                                        name="T2")
                        tt = nc.vector.tensor_tensor
                        uu = U[:, :ch_t, :]
                        t2 = T2[:, :ch_t, :]
                        tt(out=uu, in0=Gv[:, :, 0, :], in1=Gv[:, :, 1, :],
                           op=AluOp.add)
                        tt(out=t2, in0=Gv[:, :, 2, :], in1=Gv[:, :, 3, :],
                           op=AluOp.add)
                        tt(out=uu, in0=uu, in1=t2, op=AluOp.add)
                        tt(out=t2, in0=Gv[:, :, 4, :], in1=Gv[:, :, 5, :],
                           op=AluOp.add)
                        tt(out=uu, in0=uu, in1=t2, op=AluOp.add)
                        nc.sync.dma_start(
                            out=shard_rows(sshard[k], c0, ch_t),
                            in_=U[:, :ch_t, :])
                    ag_s(k, s_ag.index((t0, ct_tiles, lr0, gr0)))

                # ---- rev pass: gather msg[b2revb] into DRAM staging ----
                # (emitted after the AG_S trigger so these gathers run on
                # gpsimd while the collective moves S on the TOPSP/SDMA side)
                for c0 in range(0, NT_B, CH):
                    ch_t = min(CH, NT_B - c0)
                    Gr = gpool.tile([128, CH, H], dt.float16,
                                    tag="g2", name="Gr")
                    for t in range(ch_t):
                        nc.gpsimd.indirect_dma_start(
                            out=Gr[:, t, :], out_offset=None,
                            in_=src[:, :],
                            in_offset=bass.IndirectOffsetOnAxis(
                                ap=idxbr_sb[:, c0 + t:c0 + t + 1], axis=0),
                        )
                    nc.sync.dma_start(
                        out=shard_rows(urev_d[k], c0, ch_t),
                        in_=Gr[:, :ch_t, :])

                # ---- B phase: msg_new = relu(inp + (S[b2a]-msg[rev])@W_h) ----
                for t0, ct_tiles, lr0, gr0 in m_ag:
                    for c0 in range(t0, t0 + ct_tiles, CH):
                        ch_t = min(CH, t0 + ct_tiles - c0)
                        G2 = gpool.tile([128, CH, H], dt.float16,
                                        tag="g2", name="G2")
                        for t in range(ch_t):
                            nc.gpsimd.indirect_dma_start(
                                out=G2[:, t, :], out_offset=None,
                                in_=stab[k][:, :],
                                in_offset=bass.IndirectOffsetOnAxis(
                                    ap=idxbs_sb[:, c0 + t:c0 + t + 1], axis=0),
                            )
                        Ur = upool.tile([128, CH, H], dt.float16, tag="t2",
                                        name="Urv")
                        nc.sync.dma_start(
                            out=Ur[:, :ch_t, :],
                            in_=shard_rows(urev_d[k], c0, ch_t))
                        U = upool.tile([128, CH, H], dt.float16, tag="u",
                                       name="Ub")
                        nc.vector.tensor_tensor(
                            out=U[:, :ch_t, :], in0=G2[:, :ch_t, :],
                            in1=Ur[:, :ch_t, :], op=AluOp.subtract)
                        inp_sb = ipool.tile([128, CH, H], dt.float16,
                                            tag="inp", name="inp_b")
                        nc.sync.dma_start(
                            out=inp_sb[:, :ch_t, :],
                            in_=shard_rows(inp_d, c0, ch_t))
                        msg_sb = mpool.tile([128, CH, H], dt.float16,
                                            tag="msg", name="msg_b")
                        for t in range(ch_t):
                            uT = transpose_u(lambda a, b: U[:, t, a:b])
                            po = popool.tile([128, H], dt.float32, tag="po",
                                             name="po_b")
                            for i, (a, b) in enumerate(h_chunks):
                                w = b - a
                                nc.tensor.matmul(
                                    out=po[:, :],
                                    lhsT=uT[0:w, i * 128:i * 128 + 128],
                                    rhs=w_h[i][:, :],
                                    start=(i == 0), stop=(i == n_hc - 1))
                            nc.vector.tensor_tensor(
                                out=po[:, :], in0=po[:, :],
                                in1=inp_sb[:, t, :], op=AluOp.add)
                            nc.scalar.activation(
                                out=msg_sb[:, t, :], in_=po[:, :],
                                func=ActF.Relu)
                        nc.sync.dma_start(
                            out=shard_rows(mshard[step], c0, ch_t),
                            in_=msg_sb[:, :ch_t, :])
                    ag_msg(step, m_ag.index((t0, ct_tiles, lr0, gr0)))

            # ---------------- readout ----------------
            src = msgs[cfg.DEPTH - 1]
            for g in range(G_n):
                pm = pmpool.tile([128, H], dt.float32, tag="pm", name="pm")
                for t0 in range(0, T_g, CHA):
                    ch_t = min(CHA, T_g - t0)
                    rt0 = g * T_g + t0
                    G3 = gpool.tile([128, CHA * 6, H], dt.float16, tag="g",
                                    name="G3r")
                    for t in range(ch_t):
                        for j in range(6):
                            nc.gpsimd.indirect_dma_start(
                                out=G3[:, t * 6 + j, :],
                                out_offset=None,
                                in_=src[:, :],
                                in_offset=bass.IndirectOffsetOnAxis(
                                    ap=idxa_sb[:, (rt0 + t) * 6 + j:
                                               (rt0 + t) * 6 + j + 1],
                                    axis=0),
                            )
                    Gv = G3[:, :ch_t * 6, :].rearrange(
                        "p (c s) h -> p c s h", s=6)
                    U = upool.tile([128, CHA, H], dt.float16, tag="u",
                                   name="Ur")
                    T2 = upool.tile([128, CHA, H], dt.float16, tag="t2",
                                    name="T2r")
                    tt = nc.vector.tensor_tensor
                    uu = U[:, :ch_t, :]
                    t2 = T2[:, :ch_t, :]
                    tt(out=uu, in0=Gv[:, :, 0, :], in1=Gv[:, :, 1, :],
                       op=AluOp.add)
                    tt(out=t2, in0=Gv[:, :, 2, :], in1=Gv[:, :, 3, :],
                       op=AluOp.add)
                    tt(out=uu, in0=uu, in1=t2, op=AluOp.add)
                    tt(out=t2, in0=Gv[:, :, 4, :], in1=Gv[:, :, 5, :],
                       op=AluOp.add)
                    tt(out=uu, in0=uu, in1=t2, op=AluOp.add)

                    xa_tiles = []
                    for i, (a, b) in enumerate(fa_chunks):
                        xt = xpool.tile([b - a, CHA * 128], dt.float16,
                                        tag=f"xa{i}", name=f"xa{i}")
                        nc.sync.dma_start(
                            out=xt[:, :ch_t * 128],
                            in_=faT[a:b, rt0 * 128:(rt0 + ch_t) * 128])
                        xa_tiles.append(xt)
                    for t in range(ch_t):
                        tg = t0 + t
                        ph = popool.tile([128, H], dt.float32, tag="po",
                                         name="ph")
                        n_mm = len(wr_x) + n_hc
                        kk = 0
                        for xt, wt in zip(xa_tiles, wr_x):
                            nc.tensor.matmul(
                                out=ph[:, :],
                                lhsT=xt[:, t * 128:(t + 1) * 128],
                                rhs=wt[:, :],
                                start=(kk == 0), stop=(kk == n_mm - 1))
                            kk += 1
                        uT = transpose_u(lambda a, b: U[:, t, a:b])
                        for i, (a, b) in enumerate(h_chunks):
                            w = b - a
                            nc.tensor.matmul(
                                out=ph[:, :],
                                lhsT=uT[0:w, i * 128:i * 128 + 128],
                                rhs=wr_h[i][:, :],
                                start=(kk == 0), stop=(kk == n_mm - 1))
                            kk += 1
                        ah = mpool.tile([128, H], dt.float16, tag="ah",
                                        name="ah")
                        nc.scalar.activation(out=ah[:, :], in_=ph[:, :],
                                             func=ActF.Relu)
                        oh = mpool.tile([128, 128], dt.float16, tag="oh",
                                        name="oh")
                        rt = g * T_g + tg
                        nc.sync.dma_start(
                            out=oh[:, :],
                            in_=oneh[rt * 128:(rt + 1) * 128, :])
                        nc.tensor.matmul(
                            out=pm[:, :], lhsT=oh[:, :], rhs=ah[:, :],
                            start=(tg == 0), stop=(tg == T_g - 1))
                out_sb = mpool.tile([128, H], dt.float32, tag="osb",
                                    name="out_sb")
                nc.vector.tensor_scalar_mul(
                    out=out_sb[:, :], in0=pm[:, :],
                    scalar1=invc_sb[:, g:g + 1])
                nc.sync.dma_start(out=mol_out[g * 128:(g + 1) * 128, :],
                                  in_=out_sb[:, :])

    nc.compile()
    return nc


# ----------------------------------------------------------------------------
# Entry point
# ----------------------------------------------------------------------------
def _get_program(cfg: Cfg, T_g: int):
    key = (cfg, T_g)
    if key not in _BUILD_CACHE:
        _BUILD_CACHE[key] = build_program(cfg, T_g)
    return _BUILD_CACHE[key]


def run_on_hw(nc, in_maps, cfg, trace=False):
    from concourse.bass_utils import run_bass_kernel_spmd
    res = run_bass_kernel_spmd(nc, in_maps, list(range(cfg.NC)), trace=trace)
    return res


def assemble_output(results, cfg: Cfg):
    out = np.zeros((cfg.NM, cfg.H), np.float32)
    for c in range(cfg.NC):
        out[c * cfg.Mc:(c + 1) * cfg.Mc] = results[c]["mol_out"][:cfg.Mc]
    return out


def kernel(**inputs) -> np.ndarray:
    cfg = FULL
    in_maps, T_g = prepare_host(inputs, cfg)
    nc = _get_program(cfg, T_g)
    res = run_on_hw(nc, in_maps, cfg)
    return assemble_output(res.results, cfg)


# revision 6
# speedup vs baseline: 1.1235x; 1.0928x over previous
"""Trainium2 Bass kernel for chemprop-style MPNN (nn_Cmpd_d_MPNN_3917010174549).

Strategy (8 NeuronCores, data-parallel with replicated tables):
  - Directed-bond message table msg [NBP, 300] fp16 replicated per core via
    chunked AllGathers (overlapped with compute).
  - Atom-sum (S) formulation: per step, S[a] = sum_j msg[a2b[a,j]] is computed
    once per atom (sharded molecule-aligned, AllGathered), then the bond
    update needs only 2 gathers per bond: u = S[b2a[b]] - msg[b2revb[b]].
    This cuts SWDGE indirect-DMA instructions (the bottleneck: ~1us fixed
    cost each) from 7 to ~5.3 per bond-tile equivalent.
  - inp = f_bonds @ W_i is computed once in step 0 and stored (fp16) in DRAM;
    steps 1-2 stream it instead of recomputing the W_i matmul.
  - Readout reuses the atom-phase: S2 chunks are consumed directly from SBUF
    into the fused [f_atoms;1;S2] @ [W_o;b_o] matmul and one-hot mean pooling.

kernel(**inputs) takes the full unsharded inputs and returns [10000, 300] f32.
"""

from dataclasses import dataclass

import numpy as np


# ----------------------------------------------------------------------------
# Configuration
# ----------------------------------------------------------------------------
@dataclass(frozen=True)
class Cfg:
    NC: int = 8            # cores
    NB: int = 300001       # bonds (incl. padding bond 0)
    NA: int = 150001       # atoms (incl. padding atom 0)
    NM: int = 10000        # molecules
    H: int = 300           # hidden
    FB: int = 147          # bond feature dim
    FA: int = 133          # atom feature dim
    MAXNB: int = 6
    DEPTH: int = 3
    CH: int = 8            # bond tiles per chunk
    CHA: int = 8           # atom-slot tiles per chunk
    N_AGM: int = 1         # AllGather chunks for the message table
    N_AGS: int = 1         # AllGather chunks for the S table

    @property
    def Bs(self):          # bonds per core (multiple of 128)
        return ((self.NB + self.NC - 1) // self.NC + 127) // 128 * 128

    @property
    def NBP(self):
        return self.Bs * self.NC

    @property
    def NT_B(self):        # bond tiles per core
        return self.Bs // 128

    @property
    def Mc(self):          # molecules per core
        assert self.NM % self.NC == 0
        return self.NM // self.NC

    @property
    def G_n(self):         # 128-molecule groups per core
        return (self.Mc + 127) // 128


FULL = Cfg()

_BUILD_CACHE: dict = {}


def _chunk_tiles(n_tiles, n_chunks):
    """Split n_tiles into n_chunks tile-counts (near equal)."""
    base = n_tiles // n_chunks
    rem = n_tiles % n_chunks
    return [base + (1 if i < rem else 0) for i in range(n_chunks)]


# ----------------------------------------------------------------------------
# Host-side preprocessing
# ----------------------------------------------------------------------------
def prepare_host(inputs: dict, cfg: Cfg):
    """Build per-core input maps. Returns (in_maps, T_g)."""
    f_atoms = np.asarray(inputs["f_atoms"], np.float32)
    f_bonds = np.asarray(inputs["f_bonds"], np.float32)
    a2b = np.asarray(inputs["a2b"], np.int32)
    b2a = np.asarray(inputs["b2a"], np.int32)
    b2revb = np.asarray(inputs["b2revb"], np.int32)
    amol = np.asarray(inputs["atom_mol_id"], np.int32)
    W_i = np.asarray(inputs["W_i"], np.float32)
    W_h = np.asarray(inputs["W_h"], np.float32)
    W_o = np.asarray(inputs["W_o"], np.float32)
    b_o = np.asarray(inputs["b_o"], np.float32)

    NC, NB, NA, NM, H, FB, FA = (
        cfg.NC, cfg.NB, cfg.NA, cfg.NM, cfg.H, cfg.FB, cfg.FA)
    Bs, NT_B, Mc, G_n = cfg.Bs, cfg.NT_B, cfg.Mc, cfg.G_n

    # ---- molecule-aligned atom slots (shared by steps' S phase + readout) ----
    mol_of_atom = amol
    counts = np.bincount(mol_of_atom, minlength=NM + 1)[:NM]
    atom_ids = np.arange(NA, dtype=np.int64)
    valid = atom_ids != 0
    order = atom_ids[valid][np.argsort(mol_of_atom[valid], kind="stable")]
    mol_sorted = mol_of_atom[order]
    starts = np.searchsorted(mol_sorted, np.arange(NM + 1))

    def group_atom_count(c, g):
        m0 = c * Mc + g * 128
        m1 = min(c * Mc + (g + 1) * 128, (c + 1) * Mc)
        return starts[m1] - starts[m0]

    T_g = 1
    for c in range(NC):
        for g in range(G_n):
            T_g = max(T_g, (group_atom_count(c, g) + 127) // 128)
    NRT = G_n * T_g                     # slot tiles per core
    SLOTS = NRT * 128                   # atom slots per core

    inv_counts = np.zeros(NM, np.float32)
    nz = counts > 0
    inv_counts[nz] = 1.0 / counts[nz]

    # slot assignment per core
    slot_atom = np.zeros((NC, SLOTS), np.int64)
    slot_valid = np.zeros((NC, SLOTS), bool)
    for c in range(NC):
        for g in range(G_n):
            m0 = c * Mc + g * 128
            m1 = min(c * Mc + (g + 1) * 128, (c + 1) * Mc)
            a0, a1 = starts[m0], starts[m1]
            n = a1 - a0
            base = g * T_g * 128
            slot_atom[c, base:base + n] = order[a0:a1]
            slot_valid[c, base:base + n] = True
    # atom 0 (padding atom, but queried via b2a) -> first free slot on core 0
    free0 = np.where(~slot_valid[0])[0]
    assert len(free0) > 0, "no free slot for atom 0"
    a0slot = free0[0]
    slot_atom[0, a0slot] = 0
    slot_valid[0, a0slot] = True

    # global S-table row per atom (S table layout: [chunk][core][rows])
    s_chunks = _chunk_tiles(NRT, cfg.N_AGS)
    s_off = np.cumsum([0] + s_chunks[:-1]) * 128      # local row offsets
    s_csz = np.array(s_chunks) * 128
    s_gbase = np.cumsum([0] + list(s_csz[:-1] * NC))  # global chunk bases

    def srow_of(c, s):
        """Global S-table row for (core c, slot s). Vectorized over s."""
        j = np.searchsorted(s_off, s, side="right") - 1
        return s_gbase[j] + c * s_csz[j] + (s - s_off[j])

    srow_atom = np.zeros(NA, np.int64)
    for c in range(NC):
        sl = np.where(slot_valid[c])[0]
        srow_atom[slot_atom[c, sl]] = srow_of(c, sl)

    # ---- bond shard + new global message-row numbering ----
    # core c owns original bonds [c*Bs, (c+1)*Bs) (with padding at the end);
    # message table layout: [chunk][core][local rows]
    m_chunks = _chunk_tiles(NT_B, cfg.N_AGM)
    m_off = np.cumsum([0] + m_chunks[:-1]) * 128
    m_csz = np.array(m_chunks) * 128
    m_gbase = np.cumsum([0] + list(m_csz[:-1] * NC))

    local_r = np.arange(Bs, dtype=np.int64)
    jj = np.searchsorted(m_off, local_r, side="right") - 1
    newrow_local = m_gbase[jj] + (local_r - m_off[jj])  # + c*m_csz[j]
    newrow = np.zeros(cfg.NBP, np.int64)
    for c in range(NC):
        o = np.minimum((c + 1) * Bs, cfg.NBP)
        n = o - c * Bs
        newrow[c * Bs:o] = newrow_local[:n] + (m_csz[jj[:n]] * c)
    # original bond id -> new global msg row
    msgrow_of_bond = newrow[:NB]

    # ---- per-core index arrays ----
    # A-phase queries: for slot tile t, neighbor j: msg row of a2b[atom,j]
    # B-phase: srow(b2a[b]) and msgrow(b2revb[b]) per bond
    wstk_h = W_h.astype(np.float16)                    # [H, H]
    wi = W_i.astype(np.float16)                        # [FB, H]
    wr = np.vstack([W_o[:FA], b_o[None, :], W_o[FA:]]).astype(np.float16)

    faT_all = np.vstack([f_atoms.T.astype(np.float16),
                         np.ones((1, NA), np.float16)])  # [FA+1, NA]

    in_maps = []
    for c in range(NC):
        sl = slice(c * Bs, min((c + 1) * Bs, cfg.NBP))
        n_real = sl.stop - sl.start

        # bond features, transposed, padded
        fbT = np.zeros((FB, Bs), np.float16)
        fbT[:, :min(n_real, NB - c * Bs) if c * Bs < NB else 0] = (
            f_bonds[c * Bs:min((c + 1) * Bs, NB)].T.astype(np.float16))

        # A-phase idx: [128, NRT*6]
        sa = slot_atom[c]                               # [SLOTS]
        qa = msgrow_of_bond[
            np.where(slot_valid[c][:, None], a2b[sa, :6], 0)]  # [SLOTS, 6]
        idxa = (qa.reshape(NRT, 128, 6)
                .transpose(1, 0, 2)
                .reshape(128, NRT * 6)
                .astype(np.int32).copy())

        # B-phase idx: [128, NT_B] each
        b_ids = np.arange(c * Bs, c * Bs + Bs, dtype=np.int64)
        b_ids = np.minimum(b_ids, NB - 1)               # pad bonds -> bond NB-1
        qs = srow_atom[b2a[b_ids]]                      # [Bs]
        qr = msgrow_of_bond[b2revb[b_ids]]              # [Bs]
        idxb_s = (qs.reshape(NT_B, 128).T.astype(np.int32).copy())
        idxb_r = (qr.reshape(NT_B, 128).T.astype(np.int32).copy())

        # readout aux
        oneh = np.zeros((NRT * 128, 128), np.float16)
        for g in range(G_n):
            m0 = c * Mc + g * 128
            m1 = min(c * Mc + (g + 1) * 128, (c + 1) * Mc)
            a0, a1 = starts[m0], starts[m1]
            n = a1 - a0
            base = g * T_g * 128
            oneh[np.arange(base, base + n),
                 mol_sorted[a0:a1] - m0] = 1.0
        faT = faT_all[:, slot_atom[c]].copy()           # [FA+1, SLOTS]
        fa_invalid = ~slot_valid[c]
        faT[:, fa_invalid] = 0
        if c == 0:
            faT[:, a0slot] = 0                          # atom 0: no readout

        invc = np.zeros((128, G_n), np.float32)
        for g in range(G_n):
            m0 = c * Mc + g * 128
            m1 = min(c * Mc + (g + 1) * 128, (c + 1) * Mc)
            invc[: m1 - m0, g] = inv_counts[m0:m1]

        in_maps.append({
            "fbT": fbT,
            "idxa": idxa, "idxb_s": idxb_s, "idxb_r": idxb_r,
            "faT": faT, "oneh": oneh, "invc": invc,
            "wi": wi, "wh": wstk_h, "wr": wr,
        })
    return in_maps, T_g


# ----------------------------------------------------------------------------
# Device program
# ----------------------------------------------------------------------------
def build_program(cfg: Cfg, T_g: int):
    import concourse.bass as bass
    import concourse.mybir as mybir
    import concourse.tile as tile
    from concourse import bacc
    from concourse.masks import make_identity

    dt = mybir.dt
    NC, H, FB, FA = cfg.NC, cfg.H, cfg.FB, cfg.FA
    Bs, NBP, NT_B = cfg.Bs, cfg.NBP, cfg.NT_B
    G_n = cfg.G_n
    NRT = G_n * T_g
    SLOTS = NRT * 128
    CH, CHA = cfg.CH, cfg.CHA
    AluOp = mybir.AluOpType
    ActF = mybir.ActivationFunctionType
    RG = [list(range(NC))]

    m_chunks = _chunk_tiles(NT_B, cfg.N_AGM)     # msg AG chunks (tiles)
    s_chunks = _chunk_tiles(NRT, cfg.N_AGS)      # S AG chunks (tiles)

    h_chunks = []
    o = 0
    while o < H:
        h_chunks.append((o, min(o + 128, H)))
        o += 128
    n_hc = len(h_chunks)

    fb_chunks = []
    o = 0
    while o < FB:
        fb_chunks.append((o, min(o + 128, FB)))
        o += 128

    fa_chunks = []
    o = 0
    while o < FA + 1:
        fa_chunks.append((o, min(o + 128, FA + 1)))
        o += 128

    nc = bacc.Bacc(
        "TRN2", target_bir_lowering=False, debug=False,
        enable_asserts=False, num_devices=NC,
    )

    fbT = nc.dram_tensor("fbT", [FB, Bs], dt.float16, kind="ExternalInput").ap()
    idxa = nc.dram_tensor("idxa", [128, NRT * 6], dt.int32,
                          kind="ExternalInput").ap()
    idxb_s = nc.dram_tensor("idxb_s", [128, NT_B], dt.int32,
                            kind="ExternalInput").ap()
    idxb_r = nc.dram_tensor("idxb_r", [128, NT_B], dt.int32,
                            kind="ExternalInput").ap()
    faT = nc.dram_tensor("faT", [FA + 1, SLOTS], dt.float16,
                         kind="ExternalInput").ap()
    oneh = nc.dram_tensor("oneh", [SLOTS, 128], dt.float16,
                          kind="ExternalInput").ap()
    invc = nc.dram_tensor("invc", [128, G_n], dt.float32,
                          kind="ExternalInput").ap()
    wi_d = nc.dram_tensor("wi", [FB, H], dt.float16, kind="ExternalInput").ap()
    wh_d = nc.dram_tensor("wh", [H, H], dt.float16, kind="ExternalInput").ap()
    wr_d = nc.dram_tensor("wr", [FA + 1 + H, H], dt.float16,
                          kind="ExternalInput").ap()
    mol_out = nc.dram_tensor("mol_out", [G_n * 128, H], dt.float32,
                             kind="ExternalOutput").ap()

    with tile.TileContext(nc) as tc:
        with (
            tc.tile_pool(name="const", bufs=1) as cpool,
            tc.tile_pool(name="dram", bufs=1, space="DRAM") as dpool,
            tc.tile_pool(name="x", bufs=2) as xpool,
            tc.tile_pool(name="g", bufs=2) as gpool,
            tc.tile_pool(name="u", bufs=2) as upool,
            tc.tile_pool(name="ut", bufs=3) as utpool,
            tc.tile_pool(name="m", bufs=2) as mpool,
            tc.tile_pool(name="i", bufs=2) as ipool,
            tc.tile_pool(name="pt", bufs=2, space="PSUM") as ptpool,
            tc.tile_pool(name="po", bufs=2, space="PSUM") as popool,
            tc.tile_pool(name="pm", bufs=2, space="PSUM") as pmpool,
        ):
            # ---------------- DRAM internals ----------------
            f8 = dt.float8e4
            msgs = [dpool.tile([NBP, H], f8, addr_space="Shared",
                               name=f"msg{s}") for s in range(cfg.DEPTH)]
            mshard = [dpool.tile([Bs, H], f8, name=f"mshard{s}")
                      for s in range(cfg.DEPTH)]
            stab = [dpool.tile([SLOTS * NC, H], f8,
                               addr_space="Shared", name=f"stab{s}")
                    for s in range(2)]
            sshard = [dpool.tile([SLOTS, H], f8, name=f"sshard{s}")
                      for s in range(2)]
            urev_d = [dpool.tile([Bs, H], f8, name=f"urev{s}")
                      for s in range(2)]
            inp_d = dpool.tile([Bs, H], dt.float16, name="inp_d")

            # ---------------- constants ----------------
            ident = cpool.tile([128, 128], dt.float16, name="ident")
            make_identity(nc, ident[:, :])

            w_i = []
            for i, (a, b) in enumerate(fb_chunks):
                t = cpool.tile([b - a, H], dt.float16, name=f"wi{i}")
                nc.sync.dma_start(out=t[:, :], in_=wi_d[a:b, :])
                w_i.append(t)
            w_h = []
            for i, (a, b) in enumerate(h_chunks):
                t = cpool.tile([b - a, H], dt.float16, name=f"wh{i}")
                nc.sync.dma_start(out=t[:, :], in_=wh_d[a:b, :])
                w_h.append(t)
            wr_x = []
            for i, (a, b) in enumerate(fa_chunks):
                t = cpool.tile([b - a, H], dt.float16, name=f"wrx{i}")
                nc.sync.dma_start(out=t[:, :], in_=wr_d[a:b, :])
                wr_x.append(t)
            wr_h = []
            for i, (a, b) in enumerate(h_chunks):
                t = cpool.tile([b - a, H], dt.float16, name=f"wrh{i}")
                nc.sync.dma_start(out=t[:, :], in_=wr_d[FA + 1 + a:FA + 1 + b, :])
                wr_h.append(t)

            idxa_sb = cpool.tile([128, NRT * 6], dt.int32, name="idxa_sb")
            nc.sync.dma_start(out=idxa_sb[:, :], in_=idxa[:, :])
            idxbs_sb = cpool.tile([128, NT_B], dt.int32, name="idxbs_sb")
            nc.sync.dma_start(out=idxbs_sb[:, :], in_=idxb_s[:, :])
            idxbr_sb = cpool.tile([128, NT_B], dt.int32, name="idxbr_sb")
            nc.sync.dma_start(out=idxbr_sb[:, :], in_=idxb_r[:, :])
            invc_sb = cpool.tile([128, G_n], dt.float32, name="invc_sb")
            nc.sync.dma_start(out=invc_sb[:, :], in_=invc[:, :])

            # ---------------- helpers ----------------
            def transpose_u(u_slice_fn):
                pt = ptpool.tile([128, n_hc * 128], dt.float16, tag="pt",
                                 name="pt")
                uT = utpool.tile([128, n_hc * 128], dt.float16, tag="uT",
                                 name="uT")
                for i, (a, b) in enumerate(h_chunks):
                    w = b - a
                    nc.tensor.transpose(
                        out=pt[0:w, i * 128:i * 128 + 128],
                        in_=u_slice_fn(a, b),
                        identity=ident[:, :])
                    nc.vector.tensor_copy(
                        out=uT[0:w, i * 128:i * 128 + 128],
                        in_=pt[0:w, i * 128:i * 128 + 128])
                return uT

            def shard_rows(shard, t0, ch_t):
                return shard[t0 * 128:(t0 + ch_t) * 128, :].rearrange(
                    "(c p) h -> p c h", p=128)

            def ag_chunks_of(chunks):
                """[(tile0, ntiles, local_row0, glob_row0), ...]"""
                out = []
                t0 = 0
                g0 = 0
                for ct in chunks:
                    out.append((t0, ct, t0 * 128, g0))
                    g0 += ct * 128 * NC
                    t0 += ct
                return out

            m_ag = ag_chunks_of(m_chunks)
            s_ag = ag_chunks_of(s_chunks)

            def ag_msg(step, agj):
                t0, ct, lr0, gr0 = m_ag[agj]
                nc.gpsimd.collective_compute(
                    "AllGather", AluOp.bypass, replica_groups=RG,
                    ins=[mshard[step][lr0:lr0 + ct * 128, :]],
                    outs=[msgs[step][gr0:gr0 + ct * 128 * NC, :]],
                )

            def ag_s(k, agj):
                t0, ct, lr0, gr0 = s_ag[agj]
                nc.gpsimd.collective_compute(
                    "AllGather", AluOp.bypass, replica_groups=RG,
                    ins=[sshard[k][lr0:lr0 + ct * 128, :]],
                    outs=[stab[k][gr0:gr0 + ct * 128 * NC, :]],
                )

            # ---------------- step 0: inp + msg0 ----------------
            for t0, ct_tiles, lr0, gr0 in m_ag:
                for c0 in range(t0, t0 + ct_tiles, CH):
                    ch_t = min(CH, t0 + ct_tiles - c0)
                    x_tiles = []
                    for i, (a, b) in enumerate(fb_chunks):
                        xt = xpool.tile([b - a, CH * 128], dt.float16,
                                        tag=f"x{i}", name=f"x{i}")
                        nc.sync.dma_start(
                            out=xt[:, :ch_t * 128],
                            in_=fbT[a:b, c0 * 128:(c0 + ch_t) * 128])
                        x_tiles.append(xt)
                    inp_sb = ipool.tile([128, CH, H], dt.float16, tag="inp0",
                                        name="inp_sb")
                    msg_sb = mpool.tile([128, CH, H], f8, tag="msg",
                                        name="msg_sb")
                    for t in range(ch_t):
                        po = popool.tile([128, H], dt.float32, tag="po",
                                         name="po")
                        for k, (xt, wt) in enumerate(zip(x_tiles, w_i)):
                            nc.tensor.matmul(
                                out=po[:, :],
                                lhsT=xt[:, t * 128:(t + 1) * 128],
                                rhs=wt[:, :],
                                start=(k == 0), stop=(k == len(w_i) - 1))
                        nc.scalar.activation(
                            out=inp_sb[:, t, :], in_=po[:, :], func=ActF.Copy)
                        nc.scalar.activation(
                            out=msg_sb[:, t, :], in_=po[:, :], func=ActF.Relu)
                    nc.sync.dma_start(
                        out=shard_rows(inp_d, c0, ch_t),
                        in_=inp_sb[:, :ch_t, :])
                    nc.sync.dma_start(
                        out=shard_rows(mshard[0], c0, ch_t),
                        in_=msg_sb[:, :ch_t, :])
                ag_msg(0, m_ag.index((t0, ct_tiles, lr0, gr0)))

            # ---------------- steps 1..DEPTH-1 ----------------
            for step in range(1, cfg.DEPTH):
                k = step - 1          # S table index
                src = msgs[step - 1]
                # ---- A phase: S[slot] = sum_j msg[a2b] ----
                for t0, ct_tiles, lr0, gr0 in s_ag:
                    for c0 in range(t0, t0 + ct_tiles, CHA):
                        ch_t = min(CHA, t0 + ct_tiles - c0)
                        G3 = gpool.tile([128, CHA * 6, H], f8,
                                        tag="g", name="G3")
                        for t in range(ch_t):
                            for j in range(6):
                                nc.gpsimd.indirect_dma_start(
                                    out=G3[:, t * 6 + j, :],
                                    out_offset=None,
                                    in_=src[:, :],
                                    in_offset=bass.IndirectOffsetOnAxis(
                                        ap=idxa_sb[:, (c0 + t) * 6 + j:
                                                   (c0 + t) * 6 + j + 1],
                                        axis=0),
                                )
                        Gv = G3[:, :ch_t * 6, :].rearrange(
                            "p (c s) h -> p c s h", s=6)
                        U = upool.tile([128, CHA, H], dt.float16, tag="u",
                                       name="U")
                        T2 = upool.tile([128, CHA, H], dt.float16, tag="t2",
                                        name="T2")
                        tt = nc.vector.tensor_tensor
                        uu = U[:, :ch_t, :]
                        t2 = T2[:, :ch_t, :]
                        tt(out=uu, in0=Gv[:, :, 0, :], in1=Gv[:, :, 1, :],
                           op=AluOp.add)
                        tt(out=t2, in0=Gv[:, :, 2, :], in1=Gv[:, :, 3, :],
                           op=AluOp.add)
                        tt(out=uu, in0=uu, in1=t2, op=AluOp.add)
                        tt(out=t2, in0=Gv[:, :, 4, :], in1=Gv[:, :, 5, :],
                           op=AluOp.add)
                        U8 = upool.tile([128, CHA, H], f8, tag="u8",
                                        name="U8")
                        tt(out=U8[:, :ch_t, :], in0=uu, in1=t2, op=AluOp.add)
                        nc.sync.dma_start(
                            out=shard_rows(sshard[k], c0, ch_t),
                            in_=U8[:, :ch_t, :])
                    ag_s(k, s_ag.index((t0, ct_tiles, lr0, gr0)))

                # ---- rev pass: gather msg[b2revb] into DRAM staging ----
                # (emitted after the AG_S trigger so these gathers run on
                # gpsimd while the collective moves S on the TOPSP/SDMA side)
                for c0 in range(0, NT_B, CH):
                    ch_t = min(CH, NT_B - c0)
                    Gr = gpool.tile([128, CH, H], f8,
                                    tag="g2", name="Gr")
                    for t in range(ch_t):
                        nc.gpsimd.indirect_dma_start(
                            out=Gr[:, t, :], out_offset=None,
                            in_=src[:, :],
                            in_offset=bass.IndirectOffsetOnAxis(
                                ap=idxbr_sb[:, c0 + t:c0 + t + 1], axis=0),
                        )
                    nc.sync.dma_start(
                        out=shard_rows(urev_d[k], c0, ch_t),
                        in_=Gr[:, :ch_t, :])

                # ---- B phase: msg_new = relu(inp + (S[b2a]-msg[rev])@W_h) ----
                for t0, ct_tiles, lr0, gr0 in m_ag:
                    for c0 in range(t0, t0 + ct_tiles, CH):
                        ch_t = min(CH, t0 + ct_tiles - c0)
                        G2 = gpool.tile([128, CH, H], f8,
                                        tag="g2", name="G2")
                        for t in range(ch_t):
                            nc.gpsimd.indirect_dma_start(
                                out=G2[:, t, :], out_offset=None,
                                in_=stab[k][:, :],
                                in_offset=bass.IndirectOffsetOnAxis(
                                    ap=idxbs_sb[:, c0 + t:c0 + t + 1], axis=0),
                            )
                        Ur = upool.tile([128, CH, H], f8, tag="ur8",
                                        name="Urv")
                        nc.sync.dma_start(
                            out=Ur[:, :ch_t, :],
                            in_=shard_rows(urev_d[k], c0, ch_t))
                        U = upool.tile([128, CH, H], dt.float16, tag="u",
                                       name="Ub")
                        nc.vector.tensor_tensor(
                            out=U[:, :ch_t, :], in0=G2[:, :ch_t, :],
                            in1=Ur[:, :ch_t, :], op=AluOp.subtract)
                        inp_sb = ipool.tile([128, CH, H], dt.float16,
                                            tag="inp", name="inp_b")
                        nc.sync.dma_start(
                            out=inp_sb[:, :ch_t, :],
                            in_=shard_rows(inp_d, c0, ch_t))
                        msg_sb = mpool.tile([128, CH, H], f8,
                                            tag="msg", name="msg_b")
                        for t in range(ch_t):
                            uT = transpose_u(lambda a, b: U[:, t, a:b])
                            po = popool.tile([128, H], dt.float32, tag="po",
                                             name="po_b")
                            for i, (a, b) in enumerate(h_chunks):
                                w = b - a
                                nc.tensor.matmul(
                                    out=po[:, :],
                                    lhsT=uT[0:w, i * 128:i * 128 + 128],
                                    rhs=w_h[i][:, :],
                                    start=(i == 0), stop=(i == n_hc - 1))
                            nc.vector.tensor_tensor(
                                out=po[:, :], in0=po[:, :],
                                in1=inp_sb[:, t, :], op=AluOp.add)
                            nc.scalar.activation(
                                out=msg_sb[:, t, :], in_=po[:, :],
                                func=ActF.Relu)
                        nc.sync.dma_start(
                            out=shard_rows(mshard[step], c0, ch_t),
                            in_=msg_sb[:, :ch_t, :])
                    ag_msg(step, m_ag.index((t0, ct_tiles, lr0, gr0)))

            # ---------------- readout ----------------
            src = msgs[cfg.DEPTH - 1]
            for g in range(G_n):
                pm = pmpool.tile([128, H], dt.float32, tag="pm", name="pm")
                for t0 in range(0, T_g, CHA):
                    ch_t = min(CHA, T_g - t0)
                    rt0 = g * T_g + t0
                    G3 = gpool.tile([128, CHA * 6, H], f8, tag="g",
                                    name="G3r")
                    for t in range(ch_t):
                        for j in range(6):
                            nc.gpsimd.indirect_dma_start(
                                out=G3[:, t * 6 + j, :],
                                out_offset=None,
                                in_=src[:, :],
                                in_offset=bass.IndirectOffsetOnAxis(
                                    ap=idxa_sb[:, (rt0 + t) * 6 + j:
                                               (rt0 + t) * 6 + j + 1],
                                    axis=0),
                            )
                    Gv = G3[:, :ch_t * 6, :].rearrange(
                        "p (c s) h -> p c s h", s=6)
                    U = upool.tile([128, CHA, H], dt.float16, tag="u",
                                   name="Ur")
                    T2 = upool.tile([128, CHA, H], dt.float16, tag="t2",
                                    name="T2r")
                    tt = nc.vector.tensor_tensor
                    uu = U[:, :ch_t, :]
                    t2 = T2[:, :ch_t, :]
                    tt(out=uu, in0=Gv[:, :, 0, :], in1=Gv[:, :, 1, :],
                       op=AluOp.add)
                    tt(out=t2, in0=Gv[:, :, 2, :], in1=Gv[:, :, 3, :],
                       op=AluOp.add)
                    tt(out=uu, in0=uu, in1=t2, op=AluOp.add)
                    tt(out=t2, in0=Gv[:, :, 4, :], in1=Gv[:, :, 5, :],
                       op=AluOp.add)
                    tt(out=uu, in0=uu, in1=t2, op=AluOp.add)

                    xa_tiles = []
                    for i, (a, b) in enumerate(fa_chunks):
                        xt = xpool.tile([b - a, CHA * 128], dt.float16,
                                        tag=f"xa{i}", name=f"xa{i}")
                        nc.sync.dma_start(
                            out=xt[:, :ch_t * 128],
                            in_=faT[a:b, rt0 * 128:(rt0 + ch_t) * 128])
                        xa_tiles.append(xt)
                    for t in range(ch_t):
                        tg = t0 + t
                        ph = popool.tile([128, H], dt.float32, tag="po",
                                         name="ph")
                        n_mm = len(wr_x) + n_hc
                        kk = 0
                        for xt, wt in zip(xa_tiles, wr_x):
                            nc.tensor.matmul(
                                out=ph[:, :],
                                lhsT=xt[:, t * 128:(t + 1) * 128],
                                rhs=wt[:, :],
                                start=(kk == 0), stop=(kk == n_mm - 1))
                            kk += 1
                        uT = transpose_u(lambda a, b: U[:, t, a:b])
                        for i, (a, b) in enumerate(h_chunks):
                            w = b - a
                            nc.tensor.matmul(
                                out=ph[:, :],
                                lhsT=uT[0:w, i * 128:i * 128 + 128],
                                rhs=wr_h[i][:, :],
                                start=(kk == 0), stop=(kk == n_mm - 1))
                            kk += 1
                        ah = mpool.tile([128, H], dt.float16, tag="ah",
                                        name="ah")
                        nc.scalar.activation(out=ah[:, :], in_=ph[:, :],
                                             func=ActF.Relu)
                        oh = mpool.tile([128, 128], dt.float16, tag="oh",
                                        name="oh")
                        rt = g * T_g + tg
                        nc.sync.dma_start(
                            out=oh[:, :],
                            in_=oneh[rt * 128:(rt + 1) * 128, :])
                        nc.tensor.matmul(
                            out=pm[:, :], lhsT=oh[:, :], rhs=ah[:, :],
                            start=(tg == 0), stop=(tg == T_g - 1))
                out_sb = mpool.tile([128, H], dt.float32, tag="osb",
                                    name="out_sb")
                nc.vector.tensor_scalar_mul(
                    out=out_sb[:, :], in0=pm[:, :],
                    scalar1=invc_sb[:, g:g + 1])
                nc.sync.dma_start(out=mol_out[g * 128:(g + 1) * 128, :],
                                  in_=out_sb[:, :])

    nc.compile()
    return nc


# ----------------------------------------------------------------------------
# Entry point
# ----------------------------------------------------------------------------
def _get_program(cfg: Cfg, T_g: int):
    key = (cfg, T_g)
    if key not in _BUILD_CACHE:
        _BUILD_CACHE[key] = build_program(cfg, T_g)
    return _BUILD_CACHE[key]


def run_on_hw(nc, in_maps, cfg, trace=False):
    from concourse.bass_utils import run_bass_kernel_spmd
    res = run_bass_kernel_spmd(nc, in_maps, list(range(cfg.NC)), trace=trace)
    return res


def assemble_output(results, cfg: Cfg):
    out = np.zeros((cfg.NM, cfg.H), np.float32)
    for c in range(cfg.NC):
        out[c * cfg.Mc:(c + 1) * cfg.Mc] = results[c]["mol_out"][:cfg.Mc]
    return out


def kernel(**inputs) -> np.ndarray:
    cfg = FULL
    in_maps, T_g = prepare_host(inputs, cfg)
    nc = _get_program(cfg, T_g)
    res = run_on_hw(nc, in_maps, cfg)
    return assemble_output(res.results, cfg)
